# revision 6
# baseline (speedup 1.0000x reference)
"""Trainium2 Bass kernel for nn_BartPooler_53815940219079 (segment_reduce).

Computes, for each of B*T segments of a [B, S, H] hidden-state tensor:
  feat = concat([segment_max, segment_mean])  -> tanh(feat @ W.T + b)

Strategy (8 NeuronCores, SPMD — one program, per-core data):
  * Host compacts each segment's used tokens into a per-core token stream
    (bf16), padding every segment with duplicates of its first token so that
    each segment occupies a whole number of G-token "groups" (plus a
    compensation group whose negative membership weight cancels the duplicate
    tokens in the sum).  Segments are dealt snake-wise across cores by size
    so all cores share one static layout.
  * Device, per 128-group tile: grouped max/sum over G=4 tokens on VectorE
    (bf16, full tree for the sum); per-segment means accumulate on TensorE
    directly in transposed [h, slot] layout (lhsT = group sums, rhs =
    membership weights carrying 1/cnt) so no epilogue transpose is needed;
    PE transposes of the max partials; per-segment max reduce on VectorE;
    then a fused [2H] x [2H, D] GEMM (bf16, 4-up column packing + fp32 fold)
    with bias + tanh in fp32.
"""

import numpy as np
import ml_dtypes

import concourse.bacc as bacc
import concourse.mybir as mybir
import concourse.tile as tile
from concourse.bass_utils import run_bass_kernel_spmd
from concourse.masks import make_identity
from concourse.tile import add_dep_helper

NCORES = 8
G = 4          # tokens per group
PTILE = 128 * G  # tokens per main tile

B, S, H, T = 16, 4096, 1024, 16
D_OUT = 1024
HB = H // 128  # h-blocks per hidden vector

F32 = mybir.dt.float32
BF16 = mybir.dt.bfloat16


def _build_schedule(parts, turns):
    """Host-side: segment list -> per-core compacted layout (uniform shapes)."""
    Bn, Tn = parts.shape
    segs = []  # (global_row, example, start_token, count)
    for b in range(Bn):
        cum = 0
        for j in range(Tn):
            c = int(parts[b, j])
            if j < int(turns[b]):
                segs.append((b * Tn + j, b, 1 + cum, c))
            cum += c

    # Deal segments to cores by size rank: slot j holds the 8 segments of
    # ranks [8j, 8j+8), one per core, so the uniform per-slot group count
    # L[j] (max over cores) is as tight as possible.
    order = sorted(range(len(segs)), key=lambda i: -segs[i][3])
    core_slots = [[] for _ in range(NCORES)]
    for rank, i in enumerate(order):
        core_slots[rank % NCORES].append(segs[i])
    seg_cap = max(len(s) for s in core_slots)

    def groups_needed(cnt):
        g = (cnt + G - 1) // G
        if cnt % G:
            g += 1  # at least one pure-duplicate group for the compensation
        return g

    # Uniform per-slot group counts across cores.
    L = []
    for j in range(seg_cap):
        m = 1
        for c in range(NCORES):
            if j < len(core_slots[c]):
                m = max(m, groups_needed(core_slots[c][j][3]))
        L.append(m)
    A = np.concatenate([[0], np.cumsum(L)]).astype(np.int64)  # slot -> group start
    ngroups = int(A[-1])
    ntiles = (ngroups + 127) // 128
    ntok = ngroups * G

    # Per-core token-gather indices (into flat [B*S]) and membership weights.
    tok_idx = np.full((NCORES, ntok), -1, dtype=np.int64)
    member = np.zeros((NCORES, 128, ntiles, seg_cap), dtype=np.float32)
    out_map = np.full((NCORES, seg_cap), -1, dtype=np.int64)
    for c in range(NCORES):
        for j, (grow, b, s0, cnt) in enumerate(core_slots[c]):
            out_map[c, j] = grow
            g0 = int(A[j])
            nfull, rem = divmod(cnt, G)
            base = b * S + s0
            t0 = base  # first token, used as the harmless duplicate
            pos = g0 * G
            tok_idx[c, pos:pos + cnt] = np.arange(base, base + cnt)
            pos += cnt
            npure = L[j] - nfull - (1 if rem else 0)
            r = (G - rem) % G
            if r:
                tok_idx[c, pos:pos + r] = t0
                pos += r
            if npure:
                tok_idx[c, pos:pos + npure * G] = t0
            # weights: real groups 1/cnt, pure groups -r/(npure*G*cnt)
            inv = 1.0 / cnt
            nreal = nfull + (1 if rem else 0)
            for k in range(nreal):
                g = g0 + k
                member[c, g % 128, g // 128, j] = inv
            beta = -r / (npure * G) * inv if (npure and r) else 0.0
            for k in range(npure):
                g = g0 + nreal + k
                member[c, g % 128, g // 128, j] = beta
    return {
        "core_slots": core_slots,
        "seg_cap": seg_cap,
        "L": L,
        "A": A,
        "ntiles": ntiles,
        "ntok": ntok,
        "tok_idx": tok_idx,
        "member": member,
        "out_map": out_map,
        "nrows": Bn * Tn,
    }


def _build_program(ntiles, seg_cap, A, L):
    """Emit the SPMD Bass program (identical for all cores)."""
    ngroups = int(A[-1])
    ntok = ngroups * G

    nc = bacc.Bacc("TRN2", target_bir_lowering=False, debug=False,
                   num_devices=NCORES)
    hid = nc.dram_tensor("hid", [ntok, H], BF16, kind="ExternalInput")
    mem = nc.dram_tensor("mem", [128, ntiles, seg_cap], BF16, kind="ExternalInput")
    wt = nc.dram_tensor("wt", [2 * H, D_OUT], BF16, kind="ExternalInput")
    brep = nc.dram_tensor("brep", [seg_cap, D_OUT], F32, kind="ExternalInput")
    fold = nc.dram_tensor("fold", [128, seg_cap], F32, kind="ExternalInput")
    out = nc.dram_tensor("out", [seg_cap, D_OUT], F32, kind="ExternalOutput")

    # slots' per-segment max reduce is emitted right after the last tile
    # covering them
    cover = [[] for _ in range(ntiles)]
    for j in range(seg_cap):
        cover[(int(A[j]) + int(L[j]) - 1) // 128].append(j)

    with tile.TileContext(nc) as tc:
        with (
            tc.tile_pool(name="const", bufs=1) as constp,
            tc.tile_pool(name="hidp", bufs=3) as hidp,
            tc.tile_pool(name="partial", bufs=2) as partp,
            tc.tile_pool(name="psum_tr", bufs=2, space="PSUM") as trpp,
            tc.tile_pool(name="psum_acc", bufs=1, space="PSUM") as accp,
            tc.tile_pool(name="psum_gem", bufs=2, space="PSUM") as gemp,
            tc.tile_pool(name="small", bufs=1) as smallp,
        ):
            ident = constp.tile([128, 128], BF16)
            make_identity(nc, ident[:])

            # W on the scalar-engine HWDGE queue, in 0.5MB chunks paced
            # behind the per-tile hid streams (sync queue) so the loop's
            # tile supply isn't starved of DMA bandwidth.
            wt_sb = constp.tile([128, 2 * HB, D_OUT], BF16)
            wt_view = wt[:].rearrange("(kb p) n -> p kb n", p=128)
            wt_dmas = []
            for wch in range(HB):
                wt_dmas.append(nc.scalar.dma_start(
                    out=wt_sb[:, 2 * wch:2 * wch + 2, :],
                    in_=wt_view[:, 2 * wch:2 * wch + 2, :],
                ))
            brep_sb = constp.tile([seg_cap, D_OUT], F32)
            nc.scalar.dma_start(out=brep_sb[:], in_=brep[:])
            fold_sb = constp.tile([128, seg_cap], F32)
            nc.scalar.dma_start(out=fold_sb[:], in_=fold[:])
            mem_sb = constp.tile([128, ntiles, seg_cap], BF16)
            nc.sync.dma_start(out=mem_sb[:], in_=mem[:])

            trmax = constp.tile([128, HB, ngroups], BF16)
            maxT = smallp.tile([128, seg_cap, HB], BF16)
            mean_ps = accp.tile([128, HB, seg_cap], F32, tag="acc")

            for t in range(ntiles):
                pt = min(128, ngroups - t * 128)  # groups in this tile
                ht = hidp.tile([128, G * H], BF16)
                hdma = nc.sync.dma_start(
                    out=ht[:pt, :],
                    in_=hid[t * PTILE:t * PTILE + pt * G, :]
                        .rearrange("(p g) h -> p (g h)", g=G),
                )
                if t < len(wt_dmas):
                    add_dep_helper(wt_dmas[t].ins, hdma.ins, True,
                                   "pace W chunks behind hid tiles")
                # Grouped sum tree (full, so the mean matmul sees a single
                # [pt, H] operand) and grouped max tree over G=4 tokens.
                gmax = partp.tile([128, H], BF16, tag="gmax")
                tsm2 = partp.tile([128, 2 * H], BF16, tag="tsm2")
                tsm1 = partp.tile([128, H], BF16, tag="tsm1")
                half = G // 2 * H
                nc.vector.tensor_tensor(out=tsm2[:pt], in0=ht[:pt, :half],
                                        in1=ht[:pt, half:], op=mybir.AluOpType.add)
                nc.vector.tensor_tensor(out=tsm1[:pt], in0=tsm2[:pt, :H],
                                        in1=tsm2[:pt, H:], op=mybir.AluOpType.add)
                nc.vector.tensor_tensor(out=ht[:pt, :half], in0=ht[:pt, :half],
                                        in1=ht[:pt, half:], op=mybir.AluOpType.max)
                nc.vector.tensor_tensor(out=gmax[:pt], in0=ht[:pt, :H],
                                        in1=ht[:pt, H:2 * H], op=mybir.AluOpType.max)
                # Segment means accumulate on PE directly in [h, slot]
                # layout: meansT[h, j] += sum_g tsm1[g, h] * member[g, j]
                # (weights already carry 1/cnt).  No epilogue transpose.
                # start=True zeroes the whole 2KB PSUM bank (all 8 chunk
                # regions), so only the very first matmul may carry it.
                for c in range(HB):
                    nc.tensor.matmul(
                        mean_ps[:, c, :],
                        lhsT=tsm1[:pt, c * 128:(c + 1) * 128],
                        rhs=mem_sb[:pt, t, :],
                        start=(t == 0 and c == 0),
                        stop=(t == ntiles - 1 and c == HB - 1),
                    )
                # transpose the max partials: [group, h] -> [h, group]
                trp = trpp.tile([128, H], BF16, tag="trp")
                for hb in range(HB):
                    nc.tensor.transpose(
                        trp[:, hb * 128:hb * 128 + pt],
                        gmax[:pt, hb * 128:(hb + 1) * 128],
                        ident[:pt, :pt],
                    )
                nc.scalar.copy(
                    out=trmax[:, :, t * 128:t * 128 + pt],
                    in_=trp[:].rearrange("p (b g) -> p b g", g=128)[:, :, :pt],
                )
                # per-segment max for slots fully covered by now
                for j in cover[t]:
                    a, l = int(A[j]), int(L[j])
                    nc.vector.reduce_max(
                        out=maxT[:, j, :],
                        in_=trmax[:, :, a:a + l],
                        axis=mybir.AxisListType.X,
                    )

            # means: PSUM -> SBUF (bf16, already transposed)
            meansT = smallp.tile([128, HB, seg_cap], BF16)
            nc.scalar.copy(out=meansT[:], in_=mean_ps[:])

            # GEMM: out[slot, n] = sum_k featT[k, slot] * wt[k, n].
            # The 16 k-block matmuls are packed 4-up into PE column groups
            # (M = seg_cap <= 32 each) so they stream concurrently; each
            # column group accumulates 4 k-blocks into its own partition
            # quadrant, and a final fp32 fold matmul sums the 4 quadrants.
            # Max k-blocks go first (i = 0, 1) so they can issue as soon as
            # the last reduce_max lands, while the means PSUM copy drains.
            assert seg_cap <= 32
            osb = smallp.tile([seg_cap, D_OUT], F32)
            for nh in range(2):
                nsl = slice(nh * 512, (nh + 1) * 512)
                gem_ps = gemp.tile([128, 512], F32, tag="gem")
                for i in range(4):
                    for cg in range(4):
                        kb = 2 * cg + i if i < 2 else HB + 2 * cg + i - 2
                        lhsT = (maxT[:, :, kb] if kb < HB
                                else meansT[:, kb - HB, :])
                        nc.tensor.matmul(
                            gem_ps[32 * cg:32 * cg + seg_cap, :],
                            lhsT=lhsT,
                            rhs=wt_sb[:, kb, nsl],
                            start=(i == 0),
                            stop=(i == 3),
                            tile_position=(0, 32 * cg),
                        )
                gem_sb = smallp.tile([128, 512], F32, tag=f"gsb{nh}")
                nc.scalar.copy(out=gem_sb[:], in_=gem_ps[:])
                fold_ps = gemp.tile([seg_cap, 512], F32, tag="fold")
                nc.tensor.matmul(fold_ps[:], lhsT=fold_sb[:, :seg_cap],
                                 rhs=gem_sb[:], start=True, stop=True)
                nc.vector.tensor_add(out=osb[:, nsl], in0=fold_ps[:],
                                     in1=brep_sb[:, nsl])
            nc.scalar.activation(osb[:], osb[:],
                                 mybir.ActivationFunctionType.Tanh)
            nc.sync.dma_start(out=out[:], in_=osb[:])

    nc.compile()
    return nc


def _build_in_maps(sched, hidden_states, W, b):
    seg_cap, ntiles = sched["seg_cap"], sched["ntiles"]
    flat = np.ascontiguousarray(
        np.asarray(hidden_states, dtype=np.float32)).reshape(B * S, H)
    wt_np = np.ascontiguousarray(
        np.asarray(W, dtype=np.float32).T.astype(ml_dtypes.bfloat16))  # [2H, D]
    brep_np = np.ascontiguousarray(
        np.broadcast_to(np.asarray(b, dtype=np.float32), (seg_cap, D_OUT)))
    fold_np = np.zeros((128, seg_cap), dtype=np.float32)
    for cg in range(4):
        for j in range(seg_cap):
            fold_np[32 * cg + j, j] = 1.0

    in_maps = []
    for c in range(NCORES):
        idx = sched["tok_idx"][c]
        stream = np.zeros((sched["ntok"], H), dtype=ml_dtypes.bfloat16)
        valid = idx >= 0
        stream[valid] = flat[idx[valid]].astype(ml_dtypes.bfloat16)
        memc = np.ascontiguousarray(
            sched["member"][c].reshape(128, ntiles, seg_cap)
            .astype(ml_dtypes.bfloat16))
        in_maps.append({
            "hid": stream,
            "mem": memc,
            "wt": wt_np,
            "brep": brep_np,
            "fold": fold_np,
        })
    return in_maps


def kernel(hidden_states, W, b, turns, parts):
    parts = np.asarray(parts)
    turns = np.asarray(turns)

    sched = _build_schedule(parts, turns)
    nc = _build_program(sched["ntiles"], sched["seg_cap"],
                        sched["A"], sched["L"])
    in_maps = _build_in_maps(sched, hidden_states, W, b)

    res = run_bass_kernel_spmd(nc, in_maps, list(range(NCORES)))

    full = np.zeros((sched["nrows"], D_OUT), dtype=np.float32)
    for c in range(NCORES):
        oc = res.results[c]["out"]
        for j in range(sched["seg_cap"]):
            g = sched["out_map"][c, j]
            if g >= 0:
                full[g] = oc[j]
    return full
